# revision 27
# baseline (speedup 1.0000x reference)
"""MoE gate (group-limited top-k routing) as a Bass/Tile kernel for 8 TRN2 cores.

Computes, per token:
  logits = hidden @ W            (K=7168, E=256)
  scores = sigmoid(logits) + bias
  group-limited routing: top-2-sum per group of 32 -> top-4 groups of 8
  top-8 of masked scores, renormalized, * 2.5

Sharding: data-parallel over tokens (1024 tokens/core), W + bias replicated.

Host-side staging (inside `kernel()`, part of the sharding strategy):
  * hidden and W are cast to fp16 -- halves HBM traffic; fp16 input rounding
    keeps the final l2 error ~1.8e-4 vs the fp32 reference (logit err ~7e-4
    abs vs 1.7 logit std); products and PSUM accumulation are exact.
  * each core's hidden shard is staged TRANSPOSED and block-major
    [2 blocks, 7168, 512 tokens], so K-chunks load straight into the matmul
    lhsT layout (no PE transposes / PSUM copybacks) with contiguous 1KB DMA
    descriptor rows.
  * the top-8 output leaves the device in its SBUF layout [128, 64] (256B
    rows) and is unstaged to [1024, 8] on the host.

Device schedule: both HWDGE queues (sync + scalar) stream 7-chunk units
k-ordered (W unit and hidden unit of the same k-range on opposite queues);
a DMA_DIRECT2D occupies its queue for ~the transfer and the two queues
together sustain ~370-440GB/s, so the 18.4MB fp16 input is a ~45-50us
floor, balanced against ~50us of fp16 matmuls (448 x ~110-130ns
LDWEIGHTS+MATMUL).  Warm-up matmuls on scratch data cover the initial DMA
fill: tensor-engine idle gaps reset the PE power ramp (HAM drops to K=4/8
half-array; this alone cost ~20% in earlier revisions).  Matmuls run
chunk-major across each 4-tile block; the last chunks of each block run
tile-major so the 4 routing epilogues start staggered, and block 0's
epilogues overlap block 1's matmuls.  Each token tile accumulates into its
own PSUM bank (a 2KB PSUM zero region supports only ONE pending
accumulation group; 8 banks).  The epilogue pipelines across four engines:
scalar (sigmoid, top8-sum via activation accum_out, final scale via
per-partition activation scale), gpsimd (bias add, group mask, masked
scores), vector (max8 group top-2 / top-4 / top-8, reciprocal).

NOTE: dma_start_transpose (XBAR) was tried and abandoned: its completion
semaphore fires before the data lands and bursts of transposes drop
descriptor groups outright.  PE transposes cost ~107ns per 128x128 chunk
(LDWEIGHTS-bound, ~48us for this shape) -- hence the host-side transpose.
"""

import sys

if "/opt/trn_rl_repo" not in sys.path:
    sys.path.insert(0, "/opt/trn_rl_repo")

import numpy as np

import concourse.bacc as bacc
import concourse.bass as bass
import concourse.mybir as mybir
import concourse.tile as tile
from concourse import bass_utils

P = 128
TOP_K = 8
N_GROUP = 8
TOPK_GROUP = 4
SCALE = 2.5

N_CORES = 8
TOKENS = 8192
HIDDEN = 7168
EXPERTS = 256

NBLK = 2            # token blocks per core
TPB = 512           # tokens per block
TTB = TPB // P      # token tiles per block (4)
KTAIL = 48          # chunks [KTAIL:] run tile-major so the block's
                    # routing epilogues start staggered
WARMUP = 25         # scratch matmuls covering the DMA fill (PE stays hot)


def build_moe_gate(
    tokens_per_core=TOKENS // N_CORES,
    hidden=HIDDEN,
    n_experts=EXPERTS,
):
    KC = hidden // P           # K-chunks of 128 (56)
    TT = tokens_per_core // P  # token tiles of 128 (8)
    GS = n_experts // N_GROUP  # experts per group (32)
    f32 = mybir.dt.float32
    f16 = mybir.dt.float16

    nc = bacc.Bacc("TRN2", target_bir_lowering=False, debug=False)
    hsT = nc.dram_tensor(
        "hidden_T", [NBLK, hidden, TPB], f16, kind="ExternalInput"
    ).ap()
    wk = nc.dram_tensor("kernel", [hidden, n_experts], f16, kind="ExternalInput").ap()
    bias = nc.dram_tensor(
        "e_score_correction_bias", [n_experts], f32, kind="ExternalInput"
    ).ap()
    out = nc.dram_tensor(
        "topk_out", [P, TT * TOP_K], f32, kind="ExternalOutput"
    ).ap()

    hsT_view = hsT.rearrange("bl (kc q) t -> bl q kc t", q=P)
    wk_view = wk.rearrange("(kc p) e -> p kc e", p=P)

    with tile.TileContext(nc) as tc:
        with (
            tc.tile_pool(name="const", bufs=1) as cpool,
            tc.tile_pool(name="hblk", bufs=2) as hbpool,
            tc.tile_pool(name="plog", bufs=1, space="PSUM") as plpool,
            tc.tile_pool(name="route", bufs=2) as rpool,
        ):
            # scratch operands for the PE warm-up (must be initialized)
            wu_a = cpool.tile([P, P], f16)
            nc.gpsimd.memset(wu_a, 0.0)
            wu_b = cpool.tile([P, n_experts], f16)
            nc.gpsimd.memset(wu_b, 0.0)

            wsb = cpool.tile([P, KC, n_experts], f16)
            bias_sb = cpool.tile([P, n_experts], f32)
            bias_bcast = bass.AP(
                tensor=bias.tensor, offset=bias.offset, ap=[[0, P]] + list(bias.ap)
            )
            nc.sync.dma_start(out=bias_sb, in_=bias_bcast)

            wout_all = cpool.tile([P, TT, TOP_K], f32)
            s25 = cpool.tile([P, TT], f32)   # per-tile 2.5/sum scales
            dsum = cpool.tile([P, TT], f32)  # per-tile top8 sums / 2.5
            sc_junk = cpool.tile([P, TOP_K], f32)

            def dma_w(eng, k0, k1):
                eng.dma_start(out=wsb[:, k0:k1, :], in_=wk_view[:, k0:k1, :])

            def dma_h(eng, hsb, bl, k0, k1):
                eng.dma_start(out=hsb[:, k0:k1, :], in_=hsT_view[bl, :, k0:k1, :])

            def load_block(bl, hsb, with_w):
                # 7-chunk units, k-ordered, alternating queues; for block 0
                # the matching W unit rides the opposite queue and the
                # leading units are split finer to cut the pipeline fill
                if with_w:
                    dma_w(nc.sync, 0, 3)
                    dma_h(nc.scalar, hsb, bl, 0, 3)
                    dma_w(nc.scalar, 3, 7)
                    dma_h(nc.sync, hsb, bl, 3, 7)
                    for u in range(1, 8):
                        k0, k1 = u * 7, (u + 1) * 7
                        qa, qb = (
                            (nc.sync, nc.scalar)
                            if u % 2 == 0
                            else (nc.scalar, nc.sync)
                        )
                        dma_w(qa, k0, k1)
                        dma_h(qb, hsb, bl, k0, k1)
                else:
                    for u in range(8):
                        k0, k1 = u * 7, (u + 1) * 7
                        eng = nc.sync if u % 2 == 0 else nc.scalar
                        dma_h(eng, hsb, bl, k0, k1)

            hsb0 = hbpool.tile([P, KC, TPB], f16, name="hsb")
            load_block(0, hsb0, True)
            pending = {0: hsb0}

            lg_all = [
                plpool.tile([P, n_experts], f32, name=f"lg{i}") for i in range(TT)
            ]

            # PE warm-up: scratch matmuls run back-to-back during the DMA
            # fill so the power ramp (HAM) is at full K/clock when the real
            # stream starts; each opens+closes its own accumulation group
            for _ in range(WARMUP):
                nc.tensor.matmul(
                    lg_all[TT - 1], lhsT=wu_a, rhs=wu_b, start=True, stop=True
                )

            for bl in range(NBLK):
                hsb = pending.pop(bl)
                if bl + 1 < NBLK:
                    nxt = hbpool.tile([P, KC, TPB], f16, name="hsb")
                    load_block(bl + 1, nxt, False)
                    pending[bl + 1] = nxt

                lg = lg_all[bl * TTB : (bl + 1) * TTB]

                def mm(k, j):
                    nc.tensor.matmul(
                        lg[j],
                        lhsT=hsb[:, k, j * P : (j + 1) * P],
                        rhs=wsb[:, k, :],
                        start=(k == 0),
                        stop=(k == KC - 1),
                    )

                # chunk-major body (follows the k-ordered arrival stream)
                for k in range(KTAIL):
                    for j in range(TTB):
                        mm(k, j)

                # tile-major tail staggers the epilogues
                for j in range(TTB):
                    for k in range(KTAIL, KC):
                        mm(k, j)

                    # ---- routing epilogue (tokens on partitions) ----
                    t = bl * TTB + j
                    sc = rpool.tile([P, n_experts], f32)
                    nc.scalar.activation(
                        sc, lg[j], mybir.ActivationFunctionType.Sigmoid
                    )
                    nc.gpsimd.tensor_add(sc, sc, bias_sb)

                    # top-2 sum per group of GS experts
                    m8 = rpool.tile([P, N_GROUP * 8], f32)
                    for g in range(N_GROUP):
                        nc.vector.max(
                            m8[:, g * 8 : (g + 1) * 8],
                            sc[:, g * GS : (g + 1) * GS],
                        )
                    m8v = m8.rearrange("p (g k) -> p g k", k=8)
                    gsum = rpool.tile([P, N_GROUP], f32)
                    nc.vector.tensor_add(gsum, m8v[:, :, 0], m8v[:, :, 1])

                    # top-TOPK_GROUP groups -> 0/1 mask via threshold
                    gmax = rpool.tile([P, 8], f32)
                    nc.vector.max(gmax, gsum)
                    gmask = rpool.tile([P, N_GROUP], f32)
                    nc.gpsimd.tensor_scalar(
                        gmask,
                        gsum,
                        gmax[:, TOPK_GROUP - 1 : TOPK_GROUP],
                        None,
                        op0=mybir.AluOpType.is_ge,
                    )

                    # the final top-8 lies within the per-group top-8s, so
                    # mask m8 (64 values) instead of the 256-wide scores
                    masked = rpool.tile([P, N_GROUP * 8], f32)
                    nc.gpsimd.tensor_mul(
                        masked.rearrange("p (g e) -> p g e", g=N_GROUP),
                        m8v,
                        gmask[:, :, None].broadcast_to([P, N_GROUP, 8]),
                    )

                    top8 = rpool.tile([P, TOP_K], f32)
                    nc.vector.max(top8, masked)

                    # dsum = sum(top8)/2.5 via activation accum; wout =
                    # top8 * (2.5/sum) via per-partition activation scale
                    nc.scalar.activation(
                        sc_junk,
                        top8,
                        mybir.ActivationFunctionType.Copy,
                        scale=1.0 / SCALE,
                        accum_out=dsum[:, t : t + 1],
                    )
                    nc.vector.reciprocal(s25[:, t : t + 1], dsum[:, t : t + 1])
                    nc.scalar.activation(
                        wout_all[:, t, :],
                        top8,
                        mybir.ActivationFunctionType.Copy,
                        scale=s25[:, t : t + 1],
                    )

            nc.sync.dma_start(
                out=out, in_=wout_all.rearrange("p t k -> p (t k)")
            )

    nc.compile()
    return nc


_CACHE = {}


def _built_nc():
    if "nc" not in _CACHE:
        _CACHE["nc"] = build_moe_gate()
    return _CACHE["nc"]


def _stage_core_hidden(hs16_core):
    # [1024, 7168] -> block-major transposed [2, 7168, 512], C-contiguous
    return np.ascontiguousarray(
        hs16_core.reshape(NBLK, TPB, HIDDEN).transpose(0, 2, 1)
    )


def _unstage_core_out(o):
    # [128, 64] -> [1024, 8]
    return o.reshape(P, TOKENS // N_CORES // P, TOP_K).transpose(1, 0, 2).reshape(
        -1, TOP_K
    )


def kernel(hidden_states, kernel, e_score_correction_bias):
    hs = np.ascontiguousarray(np.asarray(hidden_states, dtype=np.float32))
    wk = np.ascontiguousarray(np.asarray(kernel, dtype=np.float32))
    bi = np.ascontiguousarray(np.asarray(e_score_correction_bias), dtype=np.float32)
    assert hs.shape == (TOKENS, HIDDEN) and wk.shape == (HIDDEN, EXPERTS)

    # stage the device shards in fp16, transposed block-major (see module doc)
    hs16 = hs.astype(np.float16)
    wk16 = wk.astype(np.float16)

    tpc = TOKENS // N_CORES
    nc = _built_nc()
    in_maps = [
        {
            "hidden_T": _stage_core_hidden(hs16[i * tpc : (i + 1) * tpc]),
            "kernel": wk16,
            "e_score_correction_bias": bi,
        }
        for i in range(N_CORES)
    ]
    res = bass_utils.run_bass_kernel_spmd(nc, in_maps, core_ids=list(range(N_CORES)))
    return np.concatenate(
        [_unstage_core_out(res.results[i]["topk_out"]) for i in range(N_CORES)],
        axis=0,
    )


# revision 28
# speedup vs baseline: 1.0162x; 1.0162x over previous
"""MoE gate (group-limited top-k routing) as a Bass/Tile kernel for 8 TRN2 cores.

Computes, per token:
  logits = hidden @ W            (K=7168, E=256)
  scores = sigmoid(logits) + bias
  group-limited routing: top-2-sum per group of 32 -> top-4 groups of 8
  top-8 of masked scores, renormalized, * 2.5

Sharding: data-parallel over tokens (1024 tokens/core), W + bias replicated.

Host-side staging (inside `kernel()`, part of the sharding strategy):
  * hidden and W are cast to fp16 -- halves HBM traffic; fp16 input rounding
    keeps the final l2 error ~1.8e-4 vs the fp32 reference (logit err ~7e-4
    abs vs 1.7 logit std); products and PSUM accumulation are exact.
  * each core's hidden shard is staged TRANSPOSED and block-major
    [2 blocks, 7168, 512 tokens], so K-chunks load straight into the matmul
    lhsT layout (no PE transposes / PSUM copybacks) with contiguous 1KB DMA
    descriptor rows.
  * the top-8 output leaves the device in its SBUF layout [128, 64] (256B
    rows) and is unstaged to [1024, 8] on the host.

Device schedule: both HWDGE queues (sync + scalar) stream 7-chunk units
k-ordered (W unit and hidden unit of the same k-range on opposite queues);
a DMA_DIRECT2D occupies its queue for ~the transfer and the two queues
together sustain ~370-440GB/s, so the 18.4MB fp16 input is a ~45-50us
floor, balanced against ~50us of fp16 matmuls (448 x ~110-130ns
LDWEIGHTS+MATMUL).  Warm-up matmuls on scratch data cover the initial DMA
fill: tensor-engine idle gaps reset the PE power ramp (HAM drops to K=4/8
half-array; this alone cost ~20% in earlier revisions).  Matmuls run
chunk-major across each 4-tile block; the last chunks of each block run
tile-major so the 4 routing epilogues start staggered, and block 0's
epilogues overlap block 1's matmuls.  Each token tile accumulates into its
own PSUM bank (a 2KB PSUM zero region supports only ONE pending
accumulation group; 8 banks).  The epilogue pipelines across four engines:
scalar (sigmoid, top8-sum via activation accum_out, final scale via
per-partition activation scale), gpsimd (bias add, group mask, masked
scores), vector (max8 group top-2 / top-4 / top-8, reciprocal).

NOTE: dma_start_transpose (XBAR) was tried and abandoned: its completion
semaphore fires before the data lands and bursts of transposes drop
descriptor groups outright.  PE transposes cost ~107ns per 128x128 chunk
(LDWEIGHTS-bound, ~48us for this shape) -- hence the host-side transpose.
"""

import sys

if "/opt/trn_rl_repo" not in sys.path:
    sys.path.insert(0, "/opt/trn_rl_repo")

import numpy as np

import concourse.bacc as bacc
import concourse.bass as bass
import concourse.mybir as mybir
import concourse.tile as tile
from concourse import bass_utils

P = 128
TOP_K = 8
N_GROUP = 8
TOPK_GROUP = 4
SCALE = 2.5

N_CORES = 8
TOKENS = 8192
HIDDEN = 7168
EXPERTS = 256

NBLK = 2            # token blocks per core
TPB = 512           # tokens per block
TTB = TPB // P      # token tiles per block (4)
KTAIL = 48          # chunks [KTAIL:] run tile-major so the block's
                    # routing epilogues start staggered
WARMUP = 25         # scratch matmuls covering the DMA fill (PE stays hot)


def build_moe_gate(
    tokens_per_core=TOKENS // N_CORES,
    hidden=HIDDEN,
    n_experts=EXPERTS,
):
    KC = hidden // P           # K-chunks of 128 (56)
    TT = tokens_per_core // P  # token tiles of 128 (8)
    GS = n_experts // N_GROUP  # experts per group (32)
    f32 = mybir.dt.float32
    f16 = mybir.dt.float16

    nc = bacc.Bacc("TRN2", target_bir_lowering=False, debug=False)
    hsT = nc.dram_tensor(
        "hidden_T", [NBLK, hidden, TPB], f16, kind="ExternalInput"
    ).ap()
    wk = nc.dram_tensor("kernel", [hidden, n_experts], f16, kind="ExternalInput").ap()
    bias = nc.dram_tensor(
        "e_score_correction_bias", [n_experts], f32, kind="ExternalInput"
    ).ap()
    out = nc.dram_tensor(
        "topk_out", [P, TT * TOP_K], f32, kind="ExternalOutput"
    ).ap()

    hsT_view = hsT.rearrange("bl (kc q) t -> bl q kc t", q=P)
    wk_view = wk.rearrange("(kc p) e -> p kc e", p=P)

    with tile.TileContext(nc) as tc:
        with (
            tc.tile_pool(name="const", bufs=1) as cpool,
            tc.tile_pool(name="hblk", bufs=2) as hbpool,
            tc.tile_pool(name="plog", bufs=1, space="PSUM") as plpool,
            tc.tile_pool(name="route", bufs=2) as rpool,
        ):
            # scratch operands for the PE warm-up (must be initialized)
            wu_a = cpool.tile([P, P], f16)
            nc.gpsimd.memset(wu_a, 0.0)
            wu_b = cpool.tile([P, n_experts], f16)
            nc.gpsimd.memset(wu_b, 0.0)

            wsb = cpool.tile([P, KC, n_experts], f16)
            bias_sb = cpool.tile([P, n_experts], f32)
            bias_bcast = bass.AP(
                tensor=bias.tensor, offset=bias.offset, ap=[[0, P]] + list(bias.ap)
            )
            nc.sync.dma_start(out=bias_sb, in_=bias_bcast)

            wout_all = cpool.tile([P, TT, TOP_K], f32)
            s25 = cpool.tile([P, TT], f32)   # per-tile 2.5/sum scales
            dsum = cpool.tile([P, TT], f32)  # per-tile top8 sums / 2.5
            sc_junk = cpool.tile([P, TOP_K], f32)

            def dma_w(eng, k0, k1):
                eng.dma_start(out=wsb[:, k0:k1, :], in_=wk_view[:, k0:k1, :])

            def dma_h(eng, hsb, bl, k0, k1):
                eng.dma_start(out=hsb[:, k0:k1, :], in_=hsT_view[bl, :, k0:k1, :])

            def load_block(bl, hsb, with_w):
                # 7-chunk units, k-ordered, alternating queues; for block 0
                # the matching W unit rides the opposite queue and the
                # leading units are split finer to cut the pipeline fill
                if with_w:
                    dma_w(nc.sync, 0, 3)
                    dma_h(nc.scalar, hsb, bl, 0, 3)
                    dma_w(nc.scalar, 3, 7)
                    dma_h(nc.sync, hsb, bl, 3, 7)
                    for u in range(1, 8):
                        k0, k1 = u * 7, (u + 1) * 7
                        qa, qb = (
                            (nc.sync, nc.scalar)
                            if u % 2 == 0
                            else (nc.scalar, nc.sync)
                        )
                        dma_w(qa, k0, k1)
                        dma_h(qb, hsb, bl, k0, k1)
                else:
                    # taper the final units so the last semaphore gates
                    # as few matmuls as possible
                    edges = [0, 7, 14, 21, 28, 35, 42, 49, 54, 56]
                    for u in range(len(edges) - 1):
                        k0, k1 = edges[u], edges[u + 1]
                        eng = nc.sync if u % 2 == 0 else nc.scalar
                        dma_h(eng, hsb, bl, k0, k1)

            hsb0 = hbpool.tile([P, KC, TPB], f16, name="hsb")
            load_block(0, hsb0, True)
            pending = {0: hsb0}

            lg_all = [
                plpool.tile([P, n_experts], f32, name=f"lg{i}") for i in range(TT)
            ]

            # PE warm-up: scratch matmuls run back-to-back during the DMA
            # fill so the power ramp (HAM) is at full K/clock when the real
            # stream starts; each opens+closes its own accumulation group
            for _ in range(WARMUP):
                nc.tensor.matmul(
                    lg_all[TT - 1], lhsT=wu_a, rhs=wu_b, start=True, stop=True
                )

            for bl in range(NBLK):
                hsb = pending.pop(bl)
                if bl + 1 < NBLK:
                    nxt = hbpool.tile([P, KC, TPB], f16, name="hsb")
                    load_block(bl + 1, nxt, False)
                    pending[bl + 1] = nxt

                lg = lg_all[bl * TTB : (bl + 1) * TTB]

                def mm(k, j):
                    nc.tensor.matmul(
                        lg[j],
                        lhsT=hsb[:, k, j * P : (j + 1) * P],
                        rhs=wsb[:, k, :],
                        start=(k == 0),
                        stop=(k == KC - 1),
                    )

                # chunk-major body (follows the k-ordered arrival stream)
                for k in range(KTAIL):
                    for j in range(TTB):
                        mm(k, j)

                # tile-major tail staggers the epilogues
                for j in range(TTB):
                    for k in range(KTAIL, KC):
                        mm(k, j)

                    # ---- routing epilogue (tokens on partitions) ----
                    t = bl * TTB + j
                    sc = rpool.tile([P, n_experts], f32)
                    nc.scalar.activation(
                        sc, lg[j], mybir.ActivationFunctionType.Sigmoid
                    )
                    nc.gpsimd.tensor_add(sc, sc, bias_sb)

                    # top-2 sum per group of GS experts
                    m8 = rpool.tile([P, N_GROUP * 8], f32)
                    for g in range(N_GROUP):
                        nc.vector.max(
                            m8[:, g * 8 : (g + 1) * 8],
                            sc[:, g * GS : (g + 1) * GS],
                        )
                    m8v = m8.rearrange("p (g k) -> p g k", k=8)
                    gsum = rpool.tile([P, N_GROUP], f32)
                    nc.vector.tensor_add(gsum, m8v[:, :, 0], m8v[:, :, 1])

                    # top-TOPK_GROUP groups -> 0/1 mask via threshold
                    gmax = rpool.tile([P, 8], f32)
                    nc.vector.max(gmax, gsum)
                    gmask = rpool.tile([P, N_GROUP], f32)
                    nc.gpsimd.tensor_scalar(
                        gmask,
                        gsum,
                        gmax[:, TOPK_GROUP - 1 : TOPK_GROUP],
                        None,
                        op0=mybir.AluOpType.is_ge,
                    )

                    # the final top-8 lies within the per-group top-8s, so
                    # mask m8 (64 values) instead of the 256-wide scores
                    masked = rpool.tile([P, N_GROUP * 8], f32)
                    nc.gpsimd.tensor_mul(
                        masked.rearrange("p (g e) -> p g e", g=N_GROUP),
                        m8v,
                        gmask[:, :, None].broadcast_to([P, N_GROUP, 8]),
                    )

                    top8 = rpool.tile([P, TOP_K], f32)
                    nc.vector.max(top8, masked)

                    # dsum = sum(top8)/2.5 via activation accum; wout =
                    # top8 * (2.5/sum) via per-partition activation scale
                    nc.scalar.activation(
                        sc_junk,
                        top8,
                        mybir.ActivationFunctionType.Copy,
                        scale=1.0 / SCALE,
                        accum_out=dsum[:, t : t + 1],
                    )
                    nc.vector.reciprocal(s25[:, t : t + 1], dsum[:, t : t + 1])
                    nc.scalar.activation(
                        wout_all[:, t, :],
                        top8,
                        mybir.ActivationFunctionType.Copy,
                        scale=s25[:, t : t + 1],
                    )

            nc.sync.dma_start(
                out=out, in_=wout_all.rearrange("p t k -> p (t k)")
            )

    nc.compile()
    return nc


_CACHE = {}


def _built_nc():
    if "nc" not in _CACHE:
        _CACHE["nc"] = build_moe_gate()
    return _CACHE["nc"]


def _stage_core_hidden(hs16_core):
    # [1024, 7168] -> block-major transposed [2, 7168, 512], C-contiguous
    return np.ascontiguousarray(
        hs16_core.reshape(NBLK, TPB, HIDDEN).transpose(0, 2, 1)
    )


def _unstage_core_out(o):
    # [128, 64] -> [1024, 8]
    return o.reshape(P, TOKENS // N_CORES // P, TOP_K).transpose(1, 0, 2).reshape(
        -1, TOP_K
    )


def kernel(hidden_states, kernel, e_score_correction_bias):
    hs = np.ascontiguousarray(np.asarray(hidden_states, dtype=np.float32))
    wk = np.ascontiguousarray(np.asarray(kernel, dtype=np.float32))
    bi = np.ascontiguousarray(np.asarray(e_score_correction_bias), dtype=np.float32)
    assert hs.shape == (TOKENS, HIDDEN) and wk.shape == (HIDDEN, EXPERTS)

    # stage the device shards in fp16, transposed block-major (see module doc)
    hs16 = hs.astype(np.float16)
    wk16 = wk.astype(np.float16)

    tpc = TOKENS // N_CORES
    nc = _built_nc()
    in_maps = [
        {
            "hidden_T": _stage_core_hidden(hs16[i * tpc : (i + 1) * tpc]),
            "kernel": wk16,
            "e_score_correction_bias": bi,
        }
        for i in range(N_CORES)
    ]
    res = bass_utils.run_bass_kernel_spmd(nc, in_maps, core_ids=list(range(N_CORES)))
    return np.concatenate(
        [_unstage_core_out(res.results[i]["topk_out"]) for i in range(N_CORES)],
        axis=0,
    )
